# revision 3
# baseline (speedup 1.0000x reference)
"""Trainium2 Bass kernel for nn_MoEBudgetAwareINLLayer (moe_routing).

The reference runs `n` steps of the linear recurrence
    error  = x - mu
    v_next = ALPHA * v - BETA * error
    x_next = x + GATE * v_next
and returns (x_n, v_n).  `h` is unused by the reference.

With e = x - mu the state [e, v] evolves by the constant 2x2 matrix
    A = [[1 - GATE*BETA, GATE*ALPHA], [-BETA, ALPHA]]
so after n steps
    x_out = p*x + q*v + (1-p)*mu
    v_out = r*x + s*v + (-r)*mu         where [[p,q],[r,s]] = A^n.

The kernel is a single elementwise pass: 3 reads + 2 writes of
(8192, 2048) f32, data-parallel over the batch dim across 8 cores
(1024 rows per core).  Per core, per [128, 2048] tile:
  ScalarE:  ax = c1*x ;  av = c4*x
  VectorE:  ax = (v*c2)+ax ; ax = (mu*c3)+ax ; av = (v*c5)+av ; av = (mu*c6)+av
"""

import numpy as np

ALPHA, BETA, GATE = 0.5, 0.1, 0.9
N_CORES = 8
B, D = 8192, 2048
ROWS = B // N_CORES  # rows per core
P = 128              # SBUF partitions
FD = D               # free-dim per tile
NT = ROWS // P       # tiles per core

_cache: dict[int, object] = {}


def _coeffs(n: int) -> tuple[float, float, float, float, float, float]:
    A = np.array(
        [[1.0 - GATE * BETA, GATE * ALPHA], [-BETA, ALPHA]], dtype=np.float64
    )
    An = np.linalg.matrix_power(A, n)
    p, q = An[0]
    r, s = An[1]
    return (float(p), float(q), float(1.0 - p), float(r), float(s), float(-r))


def _build(n: int):
    import concourse.tile as tile
    from concourse import bacc, mybir

    c1, c2, c3, c4, c5, c6 = _coeffs(n)
    mult = mybir.AluOpType.mult
    add = mybir.AluOpType.add
    dt = mybir.dt.float32

    nc = bacc.Bacc(
        "TRN2", target_bir_lowering=False, debug=False, num_devices=N_CORES
    )
    x = nc.dram_tensor("x", [ROWS, FD], dt, kind="ExternalInput").ap()
    v = nc.dram_tensor("v", [ROWS, FD], dt, kind="ExternalInput").ap()
    mu = nc.dram_tensor("mu", [ROWS, FD], dt, kind="ExternalInput").ap()
    xo = nc.dram_tensor("x_out", [ROWS, FD], dt, kind="ExternalOutput").ap()
    vo = nc.dram_tensor("v_out", [ROWS, FD], dt, kind="ExternalOutput").ap()

    xt = x.rearrange("(n p) m -> n p m", p=P)
    vt = v.rearrange("(n p) m -> n p m", p=P)
    mt = mu.rearrange("(n p) m -> n p m", p=P)
    xot = xo.rearrange("(n p) m -> n p m", p=P)
    vot = vo.rearrange("(n p) m -> n p m", p=P)

    with tile.TileContext(nc) as tc:
        with (
            tc.tile_pool(name="io", bufs=3) as iop,
            tc.tile_pool(name="acc", bufs=3) as accp,
        ):
            for i in range(NT):
                tx = iop.tile([P, FD], dt, tag="x")
                nc.sync.dma_start(tx[:], xt[i])
                tv = iop.tile([P, FD], dt, tag="v")
                nc.sync.dma_start(tv[:], vt[i])
                tm = iop.tile([P, FD], dt, tag="mu")
                nc.sync.dma_start(tm[:], mt[i])

                ax = accp.tile([P, FD], dt, tag="ax")
                av = accp.tile([P, FD], dt, tag="av")
                nc.scalar.mul(ax[:], tx[:], c1)
                nc.scalar.mul(av[:], tx[:], c4)
                nc.vector.scalar_tensor_tensor(ax[:], tv[:], c2, ax[:], mult, add)
                nc.vector.scalar_tensor_tensor(ax[:], tm[:], c3, ax[:], mult, add)
                nc.vector.scalar_tensor_tensor(av[:], tv[:], c5, av[:], mult, add)
                nc.vector.scalar_tensor_tensor(av[:], tm[:], c6, av[:], mult, add)

                nc.sync.dma_start(xot[i], ax[:])
                nc.sync.dma_start(vot[i], av[:])

    nc.compile()
    return nc


def _get_nc(n: int):
    if n not in _cache:
        _cache[n] = _build(n)
    return _cache[n]


# Test-harness hooks: test.py sets TRACE=True to profile; the raw
# BassKernelResults of the last run is stashed in LAST_RESULTS.
TRACE = False
TRACE_KWARGS: dict = {}
LAST_RESULTS = None


def kernel(h, x_init, v_init, mu, default_iterations):
    global LAST_RESULTS
    from concourse.bass_utils import run_bass_kernel_spmd

    n = int(default_iterations)
    nc = _get_nc(n)

    x_init = np.ascontiguousarray(x_init, dtype=np.float32)
    v_init = np.ascontiguousarray(v_init, dtype=np.float32)
    mu = np.ascontiguousarray(mu, dtype=np.float32)

    in_maps = [
        {
            "x": x_init[i * ROWS : (i + 1) * ROWS],
            "v": v_init[i * ROWS : (i + 1) * ROWS],
            "mu": mu[i * ROWS : (i + 1) * ROWS],
        }
        for i in range(N_CORES)
    ]
    res = run_bass_kernel_spmd(
        nc, in_maps, core_ids=list(range(N_CORES)), trace=TRACE, **TRACE_KWARGS
    )
    LAST_RESULTS = res
    x_out = np.concatenate([res.results[i]["x_out"] for i in range(N_CORES)], axis=0)
    v_out = np.concatenate([res.results[i]["v_out"] for i in range(N_CORES)], axis=0)
    return x_out, v_out


# revision 8
# speedup vs baseline: 1.0075x; 1.0075x over previous
"""Trainium2 Bass kernel for nn_MoEBudgetAwareINLLayer (moe_routing).

The reference runs `n` steps of the linear recurrence
    error  = x - mu
    v_next = ALPHA * v - BETA * error
    x_next = x + GATE * v_next
and returns (x_n, v_n).  `h` is unused by the reference.

With e = x - mu the state [e, v] evolves by the constant 2x2 matrix
    A = [[1 - GATE*BETA, GATE*ALPHA], [-BETA, ALPHA]]
so after n steps
    x_out = p*x + q*v + (1-p)*mu
    v_out = r*x + s*v + (-r)*mu         where [[p,q],[r,s]] = A^n.

The kernel is a single elementwise pass: 3 reads + 2 writes of
(8192, 2048) f32, data-parallel over the batch dim across 8 cores
(1024 rows per core).  Per core, per [128, 2048] tile:
  ScalarE:  ax = c1*x ;  av = c4*x
  VectorE:  ax = (v*c2)+ax ; ax = (mu*c3)+ax ; av = (v*c5)+av ; av = (mu*c6)+av
"""

import numpy as np

import os

ALPHA, BETA, GATE = 0.5, 0.1, 0.9
N_CORES = 8
B, D = 8192, 2048
ROWS = B // N_CORES  # rows per core
P = 128              # SBUF partitions
FD = int(os.environ.get("K_FD", D))   # free-dim per tile
NT = (ROWS * D) // (P * FD)           # tiles per core
IO_BUFS = int(os.environ.get("K_IO_BUFS", 3))
ACC_BUFS = int(os.environ.get("K_ACC_BUFS", 3))

_cache: dict[int, object] = {}


def _coeffs(n: int) -> tuple[float, float, float, float, float, float]:
    A = np.array(
        [[1.0 - GATE * BETA, GATE * ALPHA], [-BETA, ALPHA]], dtype=np.float64
    )
    An = np.linalg.matrix_power(A, n)
    p, q = An[0]
    r, s = An[1]
    return (float(p), float(q), float(1.0 - p), float(r), float(s), float(-r))


def _build(n: int):
    import concourse.tile as tile
    from concourse import bacc, mybir

    c1, c2, c3, c4, c5, c6 = _coeffs(n)
    mult = mybir.AluOpType.mult
    add = mybir.AluOpType.add
    dt = mybir.dt.float32

    nc = bacc.Bacc(
        "TRN2", target_bir_lowering=False, debug=False, num_devices=N_CORES
    )
    drows = ROWS * D // FD  # DRAM view: [drows, FD], same contiguous bytes
    x = nc.dram_tensor("x", [drows, FD], dt, kind="ExternalInput").ap()
    v = nc.dram_tensor("v", [drows, FD], dt, kind="ExternalInput").ap()
    mu = nc.dram_tensor("mu", [drows, FD], dt, kind="ExternalInput").ap()
    xo = nc.dram_tensor("x_out", [drows, FD], dt, kind="ExternalOutput").ap()
    vo = nc.dram_tensor("v_out", [drows, FD], dt, kind="ExternalOutput").ap()

    xt = x.rearrange("(n p) m -> n p m", p=P)
    vt = v.rearrange("(n p) m -> n p m", p=P)
    mt = mu.rearrange("(n p) m -> n p m", p=P)
    xot = xo.rearrange("(n p) m -> n p m", p=P)
    vot = vo.rearrange("(n p) m -> n p m", p=P)

    with tile.TileContext(nc) as tc:
        with (
            tc.tile_pool(name="io", bufs=IO_BUFS) as iop,
            tc.tile_pool(name="acc", bufs=ACC_BUFS) as accp,
        ):
            for i in range(NT):
                tx = iop.tile([P, FD], dt, tag="x")
                nc.sync.dma_start(tx[:], xt[i])
                tv = iop.tile([P, FD], dt, tag="v")
                nc.sync.dma_start(tv[:], vt[i])
                tm = iop.tile([P, FD], dt, tag="mu")
                nc.sync.dma_start(tm[:], mt[i])

                ax = accp.tile([P, FD], dt, tag="ax")
                av = accp.tile([P, FD], dt, tag="av")
                nc.scalar.mul(ax[:], tx[:], c1)
                nc.scalar.mul(av[:], tx[:], c4)
                nc.vector.scalar_tensor_tensor(ax[:], tv[:], c2, ax[:], mult, add)
                nc.vector.scalar_tensor_tensor(ax[:], tm[:], c3, ax[:], mult, add)
                nc.vector.scalar_tensor_tensor(av[:], tv[:], c5, av[:], mult, add)
                nc.vector.scalar_tensor_tensor(av[:], tm[:], c6, av[:], mult, add)

                nc.sync.dma_start(xot[i], ax[:])
                nc.sync.dma_start(vot[i], av[:])

    nc.compile()
    return nc


def _get_nc(n: int):
    if n not in _cache:
        _cache[n] = _build(n)
    return _cache[n]


# Test-harness hooks: test.py sets TRACE=True to profile; the raw
# BassKernelResults of the last run is stashed in LAST_RESULTS.
TRACE = False
TRACE_KWARGS: dict = {}
LAST_RESULTS = None


def kernel(h, x_init, v_init, mu, default_iterations):
    global LAST_RESULTS
    from concourse.bass_utils import run_bass_kernel_spmd

    n = int(default_iterations)
    nc = _get_nc(n)

    x_init = np.ascontiguousarray(x_init, dtype=np.float32)
    v_init = np.ascontiguousarray(v_init, dtype=np.float32)
    mu = np.ascontiguousarray(mu, dtype=np.float32)

    in_maps = [
        {
            "x": x_init[i * ROWS : (i + 1) * ROWS].reshape(-1, FD),
            "v": v_init[i * ROWS : (i + 1) * ROWS].reshape(-1, FD),
            "mu": mu[i * ROWS : (i + 1) * ROWS].reshape(-1, FD),
        }
        for i in range(N_CORES)
    ]
    res = run_bass_kernel_spmd(
        nc, in_maps, core_ids=list(range(N_CORES)), trace=TRACE, **TRACE_KWARGS
    )
    LAST_RESULTS = res
    x_out = np.concatenate(
        [res.results[i]["x_out"].reshape(ROWS, D) for i in range(N_CORES)], axis=0
    )
    v_out = np.concatenate(
        [res.results[i]["v_out"].reshape(ROWS, D) for i in range(N_CORES)], axis=0
    )
    return x_out, v_out


# revision 9
# speedup vs baseline: 1.1534x; 1.1447x over previous
"""Trainium2 Bass kernel for nn_MoEBudgetAwareINLLayer (moe_routing).

The reference runs `n` steps of the linear recurrence
    error  = x - mu
    v_next = ALPHA * v - BETA * error
    x_next = x + GATE * v_next
and returns (x_n, v_n).  `h` is unused by the reference.

With e = x - mu the state [e, v] evolves by the constant 2x2 matrix
    A = [[1 - GATE*BETA, GATE*ALPHA], [-BETA, ALPHA]]
so after n steps
    x_out = p*x + q*v + (1-p)*mu
    v_out = r*x + s*v + (-r)*mu         where [[p,q],[r,s]] = A^n.

The kernel is a single elementwise pass: 3 reads + 2 writes of
(8192, 2048) f32, data-parallel over the batch dim across 8 cores
(1024 rows per core).  Per core, per [128, 2048] tile:
  ScalarE:  ax = c1*x ;  av = c4*x
  VectorE:  ax = (v*c2)+ax ; ax = (mu*c3)+ax ; av = (v*c5)+av ; av = (mu*c6)+av
"""

import numpy as np

import os

ALPHA, BETA, GATE = 0.5, 0.1, 0.9
N_CORES = 8
B, D = 8192, 2048
ROWS = B // N_CORES  # rows per core
P = 128              # SBUF partitions
FD = int(os.environ.get("K_FD", D))   # free-dim per tile
NT = (ROWS * D) // (P * FD)           # tiles per core
IO_BUFS = int(os.environ.get("K_IO_BUFS", 3))
ACC_BUFS = int(os.environ.get("K_ACC_BUFS", 3))

_cache: dict[int, object] = {}


def _coeffs(n: int) -> tuple[float, float, float, float, float, float]:
    A = np.array(
        [[1.0 - GATE * BETA, GATE * ALPHA], [-BETA, ALPHA]], dtype=np.float64
    )
    An = np.linalg.matrix_power(A, n)
    p, q = An[0]
    r, s = An[1]
    return (float(p), float(q), float(1.0 - p), float(r), float(s), float(-r))


def _build(n: int):
    import concourse.tile as tile
    from concourse import bacc, mybir

    c1, c2, c3, c4, c5, c6 = _coeffs(n)
    mult = mybir.AluOpType.mult
    add = mybir.AluOpType.add
    dt = mybir.dt.float32

    nc = bacc.Bacc(
        "TRN2", target_bir_lowering=False, debug=False, num_devices=N_CORES
    )
    drows = ROWS * D // FD  # DRAM view: [drows, FD], same contiguous bytes
    x = nc.dram_tensor("x", [drows, FD], dt, kind="ExternalInput").ap()
    v = nc.dram_tensor("v", [drows, FD], dt, kind="ExternalInput").ap()
    mu = nc.dram_tensor("mu", [drows, FD], dt, kind="ExternalInput").ap()
    xo = nc.dram_tensor("x_out", [drows, FD], dt, kind="ExternalOutput").ap()
    vo = nc.dram_tensor("v_out", [drows, FD], dt, kind="ExternalOutput").ap()

    xt = x.rearrange("(n p) m -> n p m", p=P)
    vt = v.rearrange("(n p) m -> n p m", p=P)
    mt = mu.rearrange("(n p) m -> n p m", p=P)
    xot = xo.rearrange("(n p) m -> n p m", p=P)
    vot = vo.rearrange("(n p) m -> n p m", p=P)

    store_eng = nc.scalar if os.environ.get("K_STORE_ACT") else nc.sync
    xb = int(os.environ.get("K_X_BUFS", IO_BUFS))
    vb = int(os.environ.get("K_V_BUFS", IO_BUFS))
    mb = int(os.environ.get("K_MU_BUFS", IO_BUFS))
    with tile.TileContext(nc) as tc:
        with (
            tc.tile_pool(name="io", bufs=IO_BUFS) as iop,
            tc.tile_pool(name="acc", bufs=ACC_BUFS) as accp,
        ):
            for i in range(NT):
                tx = iop.tile([P, FD], dt, tag="x", bufs=xb)
                nc.sync.dma_start(tx[:], xt[i])
                tv = iop.tile([P, FD], dt, tag="v", bufs=vb)
                nc.sync.dma_start(tv[:], vt[i])
                tm = iop.tile([P, FD], dt, tag="mu", bufs=mb)
                nc.sync.dma_start(tm[:], mt[i])

                ax = accp.tile([P, FD], dt, tag="ax")
                av = accp.tile([P, FD], dt, tag="av")
                nc.scalar.mul(ax[:], tx[:], c1)
                nc.scalar.mul(av[:], tx[:], c4)
                nc.vector.scalar_tensor_tensor(ax[:], tv[:], c2, ax[:], mult, add)
                nc.vector.scalar_tensor_tensor(ax[:], tm[:], c3, ax[:], mult, add)
                nc.vector.scalar_tensor_tensor(av[:], tv[:], c5, av[:], mult, add)
                nc.vector.scalar_tensor_tensor(av[:], tm[:], c6, av[:], mult, add)

                store_eng.dma_start(xot[i], ax[:])
                store_eng.dma_start(vot[i], av[:])

    nc.compile()
    return nc


def _get_nc(n: int):
    if n not in _cache:
        _cache[n] = _build(n)
    return _cache[n]


# Test-harness hooks: test.py sets TRACE=True to profile; the raw
# BassKernelResults of the last run is stashed in LAST_RESULTS.
TRACE = False
TRACE_KWARGS: dict = {}
LAST_RESULTS = None


def kernel(h, x_init, v_init, mu, default_iterations):
    global LAST_RESULTS
    from concourse.bass_utils import run_bass_kernel_spmd

    n = int(default_iterations)
    nc = _get_nc(n)

    x_init = np.ascontiguousarray(x_init, dtype=np.float32)
    v_init = np.ascontiguousarray(v_init, dtype=np.float32)
    mu = np.ascontiguousarray(mu, dtype=np.float32)

    in_maps = [
        {
            "x": x_init[i * ROWS : (i + 1) * ROWS].reshape(-1, FD),
            "v": v_init[i * ROWS : (i + 1) * ROWS].reshape(-1, FD),
            "mu": mu[i * ROWS : (i + 1) * ROWS].reshape(-1, FD),
        }
        for i in range(N_CORES)
    ]
    res = run_bass_kernel_spmd(
        nc, in_maps, core_ids=list(range(N_CORES)), trace=TRACE, **TRACE_KWARGS
    )
    LAST_RESULTS = res
    x_out = np.concatenate(
        [res.results[i]["x_out"].reshape(ROWS, D) for i in range(N_CORES)], axis=0
    )
    v_out = np.concatenate(
        [res.results[i]["v_out"].reshape(ROWS, D) for i in range(N_CORES)], axis=0
    )
    return x_out, v_out
